# revision 1
# baseline (speedup 1.0000x reference)
"""BitLinear (1-bit packed weights) on 8 TRN2 NeuronCores.

out = x @ W.T, x [64, 4096] f32, W [11008, 4096] in {-1,+1} unpacked from
bp (one byte per int32, MSB-first bits).

Strategy (tensor-parallel, no collectives):
 - shard out_features 11008 -> 8 x 1376 rows of W; x replicated.
 - host: repack bp bytes into dense 16-bit words (pure bit layout change),
   transposed to [word-idx, n] and laid out as one [128, 2752] tile per
   core (both 128-word chunks side by side); permute x to match.
 - device per core (raw Block, manual semaphores):
     DVE: w1 = words & (1<<s)            (one op extracts BOTH chunks' plane)
     DVE/ACT: u = w1 * 2^(1-s) - 1       (arith + cast -> exact {-1,+1} bf16)
     PE: column-tiled pairs - chunk (c=0,o) on array cols 0-63 -> psum
         partitions 0-63, chunk (c=1,o) on cols 64-127 -> partitions 64-127,
         running concurrently; accumulate over o=0..15.
     DVE: merge psum[0:64] + psum[64:128] -> out tile; DMA out.
 - PE warmup: dummy matmuls during the input-DMA wait so HAM un-throttles
   before the real accumulation starts.
"""

import sys

sys.path.insert(0, "/opt/trn_rl_repo")

import ml_dtypes
import numpy as np

import concourse.bass as bass
import concourse.mybir as mybir
from concourse.bass_utils import run_bass_kernel_spmd

OUT_F = 11008
IN_F = 4096
M = 64
NCORES = 8
NSH = OUT_F // NCORES  # 1376 rows of W per core
NSH2 = 2 * NSH  # pair-tile width (both chunks)

PACK = 16  # bits per packed word on device
NW = IN_F // PACK  # packed words along k per W row (256)
NCH = NW // 128  # 128-partition word chunks (2)
NPAIR = PACK  # 16 plane-pairs (o = bit offset in word)
NA = NSH // 2  # 688: output columns per column-tile half
QSPLITS = (512, NA - 512)  # psum n-chunks per half (bank = 512 f32)

_dt_word = {16: mybir.dt.uint16, 32: mybir.dt.uint32}[PACK]
_np_word = {16: "<u2", 32: "<u4"}[PACK]

B1 = 5  # w1 pair buffer depth
B2 = 8  # u pair buffer depth
N_WARMUP = 34  # dummy PE matmuls (N=512) to trip the HAM un-throttle
ACT_CAST = frozenset({1, 3, 5, 7, 9, 11})  # pairs whose cast runs on ACT


def _shift(o):
    # word bit position holding k-offset o (little-endian byte packing,
    # MSB-first bit order inside each byte)
    return 8 * (o // 8) + 7 - (o % 8)


def _build():
    nc = bass.Bass()
    bpt = nc.declare_dram_parameter("bpt", [128, NSH2], _dt_word, isOutput=False)
    xr = nc.declare_dram_parameter(
        "xr", [128, (IN_F // 128) * M], mybir.dt.bfloat16, isOutput=False
    )
    out = nc.declare_dram_parameter("out", [M, NSH], mybir.dt.float32, isOutput=True)

    A = mybir.AluOpType

    # engine program-order bookkeeping
    dve_idx = {}  # ('and'|'cast', o) -> 1-based completion count on DVE
    act_idx = {}  # o -> 1-based completion count on ACT
    di = 0
    ai = 0
    for o in range(NPAIR):
        if _shift(o) == 15:
            di += 1
            dve_idx[("cast", o)] = di
            continue
        di += 1
        dve_idx[("and", o)] = di
        if o in ACT_CAST:
            ai += 1
            act_idx[o] = ai
        else:
            di += 1
            if o == NPAIR - 1:
                dve_idx[("cast_h0", o)] = di
                di += 1
            dve_idx[("cast", o)] = di

    with (
        nc.sbuf_tensor("xb", [128, (IN_F // 128) * M], mybir.dt.bfloat16) as xb,
        nc.sbuf_tensor("btw", [128, NSH2], _dt_word) as btw,
        nc.sbuf_tensor("w1", [128, B1, NSH2], _dt_word) as w1,
        nc.sbuf_tensor("u", [128, B2, NSH2], mybir.dt.bfloat16) as u,
        nc.sbuf_tensor("ot2", [128, NA], mybir.dt.float32) as ot2,
        nc.sbuf_tensor("junk", [128, 512], mybir.dt.bfloat16) as junk,
        nc.sbuf_tensor("scr", [1, 1], mybir.dt.float32) as scr,
        nc.psum_tensor("q0", [128, QSPLITS[0]], mybir.dt.float32) as q0,
        nc.psum_tensor("q1", [128, QSPLITS[1]], mybir.dt.float32) as q1,
        nc.psum_tensor("psw", [M, 512], mybir.dt.float32) as psw,
        nc.semaphore("sq") as sq,
        nc.semaphore("sb") as sb,
        nc.semaphore("sv") as sv,
        nc.semaphore("sa") as sa,
        nc.semaphore("sp") as sp,
        nc.semaphore("scp") as scp,
        nc.semaphore("so") as so,
        nc.semaphore("sdone") as sdone,
        nc.Block() as block,
    ):

        @block.gpsimd
        def _(gpsimd: bass.BassEngine):
            gpsimd.wait_ge(sb, 64)
            gpsimd.dma_start(out=xb[0:64, :], in_=xr[0:64, :]).then_inc(sq, 16)

        @block.sync
        def _(sync: bass.BassEngine):
            sync.dma_start(out=btw[32:64, :], in_=bpt[32:64, :]).then_inc(sb, 16)
            sync.dma_start(out=btw[0:32, :], in_=bpt[0:32, :]).then_inc(sb, 16)
            sync.wait_ge(sb, 64)
            sync.dma_start(out=xb[64:128, :], in_=xr[64:128, :]).then_inc(sq, 16)
            # output: two DMAs, one per column-tile half
            sync.wait_ge(sa, len(ACT_CAST) + 2)
            sync.dma_start(out=out[:, 0:NA], in_=ot2[0:M, :]).then_inc(so, 16)
            sync.wait_ge(scp, 2)
            sync.dma_start(out=out[:, NA:NSH], in_=ot2[M : 2 * M, :]).then_inc(so, 16)
            sync.wait_ge(so, 32)
            sync.wait_ge(sdone, 3)
            # (bass clears kernel sems in its own preamble on each execution)

        @block.vector
        def _(vector: bass.BassEngine):
            vector.wait_ge(sb, 64)
            for o in range(NPAIR):
                s = _shift(o)
                if s == 15:
                    # sign bit via compare: u = (v >= 2^15) - 0.5 in {-.5,+.5};
                    # the matching x blocks are pre-scaled by 2 on the host
                    if o >= B2:
                        vector.wait_ge(sp, o - B2 + 1)
                    vector.tensor_scalar(
                        u[:, o % B2, :],
                        btw[:, :],
                        32768.0,
                        0.5,
                        op0=A.is_ge,
                        op1=A.subtract,
                    ).then_inc(sv)
                    continue
                # w1 slot free? its reader is cast(o-B1)
                if o >= B1 and (o - B1) in ACT_CAST:
                    vector.wait_ge(sa, act_idx[o - B1])
                vector.tensor_scalar(
                    w1[:, o % B1, :], btw[:, :], 1 << s, None, op0=A.bitwise_and
                ).then_inc(sv)
                if o not in ACT_CAST:
                    if o >= B2:
                        vector.wait_ge(sp, o - B2 + 1)
                    if o == NPAIR - 1:
                        # halved cast: c=0 half lands one op earlier so the
                        # PE can start the final pair sooner
                        for h in range(2):
                            vector.tensor_scalar(
                                u[:, o % B2, h * NSH : (h + 1) * NSH],
                                w1[:, o % B1, h * NSH : (h + 1) * NSH],
                                float(2.0 ** (1 - s)),
                                -1.0,
                                op0=A.mult,
                                op1=A.add,
                            ).then_inc(sv)
                    else:
                        vector.tensor_scalar(
                            u[:, o % B2, :],
                            w1[:, o % B1, :],
                            float(2.0 ** (1 - s)),
                            -1.0,
                            op0=A.mult,
                            op1=A.add,
                        ).then_inc(sv)
            # copy the B column-tile halves PSUM -> SBUF
            qs = [q0, q1]
            off = 0
            for j, w in enumerate(QSPLITS):
                vector.wait_ge(sp, NPAIR + j)
                vector.tensor_copy(ot2[M : 2 * M, off : off + w], qs[j][M : 2 * M, :]).then_inc(scp)
                off += w
            vector.nop().then_inc(sdone)

        @block.scalar
        def _(scalar: bass.BassEngine):
            scalar.dma_start(out=btw[96:128, :], in_=bpt[96:128, :]).then_inc(sb, 16)
            scalar.dma_start(out=btw[64:96, :], in_=bpt[64:96, :]).then_inc(sb, 16)
            # touch the ACT path early so the activation table loads during
            # the DMA wait instead of on the first real cast
            scalar.activation(
                scr[:, :], scr[:, :], mybir.ActivationFunctionType.Copy, 0.0, 0.0
            )
            for o in sorted(ACT_CAST):
                s = _shift(o)
                scalar.wait_ge(sv, dve_idx[("and", o)])
                if o >= B2:
                    scalar.wait_ge(sp, o - B2 + 1)
                scalar.activation(
                    u[:, o % B2, :],
                    w1[:, o % B1, :],
                    mybir.ActivationFunctionType.Copy,
                    bias=-1.0,
                    scale=float(2.0 ** (1 - s)),
                ).then_inc(sa)
            # copy the A column-tile halves PSUM -> SBUF
            qs = [q0, q1]
            off = 0
            for j, w in enumerate(QSPLITS):
                scalar.wait_ge(sp, NPAIR + j)
                scalar.activation(
                    ot2[0:M, off : off + w],
                    qs[j][0:M, :],
                    mybir.ActivationFunctionType.Copy,
                    bias=0.0,
                    scale=1.0,
                ).then_inc(sa)
                off += w
            scalar.nop().then_inc(sdone)

        @block.tensor
        def _(tensor: bass.BassEngine):
            # HAM warmup on junk data (no DMA dependency)
            for _i in range(N_WARMUP):
                tensor.matmul(
                    psw[:, :], junk[:, 0:M], junk[:, :], start=True, stop=True
                )
            tensor.wait_ge(sq, 32)
            for o in range(NPAIR):
                if o in ACT_CAST:
                    tensor.wait_ge(sa, act_idx[o])
                elif o == NPAIR - 1:
                    tensor.wait_ge(sv, dve_idx[("cast_h0", o)])
                else:
                    tensor.wait_ge(sv, dve_idx[("cast", o)])
                qs = [q0, q1]
                last_pair = o == NPAIR - 1
                ins = None
                for c in range(NCH):
                    if last_pair and c == 1:
                        tensor.wait_ge(sv, dve_idx[("cast", o)])
                    lh = xb[:, (c * PACK + o) * M : (c * PACK + o + 1) * M]
                    st = o == 0 and c == 0
                    sp_ = last_pair and c == NCH - 1
                    base = c * NSH
                    # explicit weight loads for both column tiles, then
                    # non-self-loading matmuls so A and B stream concurrently
                    tensor.ldweights(lh, tile_position=(0, 0))
                    tensor.ldweights(lh, tile_position=(0, 64))
                    # tile A: output cols [0:NA] on psum partitions 0:64
                    # tile B: output cols [NA:NSH] on partitions 64:128
                    for j, w in enumerate(QSPLITS):
                        off = 512 * j
                        i1 = tensor.matmul(
                            qs[j][0:M, :],
                            lh,
                            u[:, o % B2, base + off : base + off + w],
                            start=st,
                            stop=sp_,
                            tile_position=(0, 0),
                        )
                        i1.ins.ldweights = False
                        ins = tensor.matmul(
                            qs[j][M : 2 * M, :],
                            lh,
                            u[:, o % B2, base + NA + off : base + NA + off + w],
                            start=st,
                            stop=sp_,
                            tile_position=(0, 64),
                        )
                        ins.ins.ldweights = False
                        if sp_:
                            ins.then_inc(sp)  # per-region completion
                if not last_pair:
                    ins.then_inc(sp)
                if o < NPAIR - 3:
                    for _k in range(2):
                        tensor.matmul(
                            psw[:, :], junk[:, 0:M], junk[:, :], start=True, stop=True
                        )
            tensor.nop().then_inc(sdone)

    return nc


def _prep(x, bp):
    x = np.asarray(x, dtype=np.float32)
    bp = np.asarray(bp)
    bytes_ = bp.astype(np.uint8)  # values are 0..255 by construction
    B = bytes_.reshape(OUT_F, IN_F // 8)
    # x[m, k] with k = PACK*(128*c + p) + o  ->  xh[p, (c, o, m)]
    xr4 = np.ascontiguousarray(x.reshape(M, NCH, 128, PACK).transpose(2, 1, 3, 0))
    for o in range(PACK):
        if _shift(o) == 15:
            xr4[:, :, o, :] *= 2.0
    xh = xr4.reshape(128, -1).astype(ml_dtypes.bfloat16)
    in_maps = []
    for cid in range(NCORES):
        Bc = np.ascontiguousarray(B[cid * NSH : (cid + 1) * NSH])  # [1376, 512] u8
        Wd = Bc.view(_np_word)  # [1376, NW] little-endian words
        bptT = np.ascontiguousarray(Wd.T)  # [NW=256, 1376]
        # both 128-word chunks side by side: [128, 2752]
        pair = np.concatenate([bptT[0:128, :], bptT[128:256, :]], axis=1)
        in_maps.append({"bpt": np.ascontiguousarray(pair), "xr": xh})
    return in_maps


def _run(x, bp, trace=False):
    in_maps = _prep(x, bp)
    nc = _build()
    res = run_bass_kernel_spmd(nc, in_maps, list(range(NCORES)), trace=trace)
    outs = [np.asarray(res.results[c]["out"]) for c in range(NCORES)]
    full = np.concatenate(outs, axis=1).astype(np.float32)
    return full, res


def kernel(x, bp):
    out, _ = _run(x, bp, trace=False)
    return out



# revision 25
# speedup vs baseline: 1.4453x; 1.4453x over previous
"""BitLinear (1-bit packed weights) on 8 TRN2 NeuronCores — v3.

out = x @ W.T, x [64, 4096] f32, W [11008, 4096] in {-1,+1} unpacked from
bp (one byte per int32, MSB-first bits).

Tensor-parallel over out_features: 8 x 1376 rows of W; x replicated.

v3 design (bitcast unpack):
 - DVE bitvec ops cannot dtype-cast (walrus verifier). Instead, the host
   repacks each weight bit into a bf16 EXPONENT bit position (9..14) of
   uint16 words. Then (word & (1<<s)) reinterpreted (bitcast) as bf16 is
   exactly {0, 2^(2^(s-7)-127)} — a clean one-op-per-plane unpack. The
   weird magnitude is compensated by prescaling the matching x slice by
   2^(128-2^(s-7)) on the host (products are always 2*x*bit).
 - 3 packed arrays btwA/B/C [128, 2752] u16: A carries 7 planes (pos
   15,14,13..9), B 6 (14..9), C 3 (14,13,12). Position 15 is extracted
   by the ACT engine via Sign (gives +-1 directly); the three pos-14
   planes go to GPSIMD tensor_tensor AND (one shared mask); the other 12
   ride the DVE (12 x ~1.04us chain).
 - "-1" of w=2b-1: rank-1 correction initializes psum via a
   contraction-2 matmul (c_hi+c_lo bf16 split for f32 accuracy).
 - Single large DMAs (packets spread over all 16 DMA engines), ungated.
 - u buffer holds all 16 planes: DVE never waits on PE.
 - PE: junk warmup for the p-state ramp, then corr + 16 planes in
   ascending-o order (matches producer completion), junk fillers between.
"""

import sys

sys.path.insert(0, "/opt/trn_rl_repo")

import ml_dtypes
import numpy as np

import concourse.bass as bass
import concourse.mybir as mybir
from concourse.bass_utils import run_bass_kernel_spmd

OUT_F = 11008
IN_F = 4096
M = 64
NCORES = 8
NSH = OUT_F // NCORES  # 1376 rows of W per core
NSH2 = 2 * NSH  # 2752
PACK = 16
NW = IN_F // PACK  # 256 words per W row
NCH = NW // 128  # 2 chunks
NPAIR = PACK  # 16 k-offsets per word
NA = NSH // 2  # 688
QS = (512, 176)  # psum n-splits per half

# plane table: k-offset o -> (array, bit position, producer)
# producers: 'dve', 'act' (Sign, pos must be 15), 'pool' (shared pos-14 mask)
PLANES = {
    0: ("A", 13, "dve"),
    1: ("A", 12, "dve"),
    2: ("A", 15, "isge"),
    3: ("A", 14, "dve"),
    4: ("A", 11, "dve"),
    5: ("A", 10, "dve"),
    6: ("A", 9, "dve"),
    7: ("B", 13, "dve"),
    8: ("B", 12, "dve"),
    9: ("B", 14, "dve"),
    10: ("B", 11, "dve"),
    11: ("B", 10, "dve"),
    12: ("B", 9, "dve"),
    13: ("C", 13, "dve"),
    14: ("C", 14, "dve"),
    15: ("C", 12, "dve"),
}
DVE_SEQ = list(range(NPAIR))  # all planes on DVE, ascending o

N_WARM = 12
WARM_N = 256
FILL_N = 192

_np_word = "<u2"  # test.py preflight compat


def _shift(o):
    # ORIGINAL byte packing of bp: bit position of k-offset o
    return 8 * (o // 8) + 7 - (o % 8)


def _xscale(o):
    arr, pos, prod = PLANES[o]
    if prod == "isge":
        return 2.0  # is_ge gives {0,1}
    # u_on = 2^(2^(pos-7)-127); want x'*u_on = 2x
    return float(2.0 ** (128 - (1 << (pos - 7))))


def _build():
    nc = bass.Bass()
    bpA = nc.declare_dram_parameter("bpA", [128, NSH2], mybir.dt.uint16, isOutput=False)
    bpB = nc.declare_dram_parameter("bpB", [128, NSH2], mybir.dt.uint16, isOutput=False)
    bpC = nc.declare_dram_parameter("bpC", [128, NSH2], mybir.dt.uint16, isOutput=False)
    xr = nc.declare_dram_parameter("xr", [128, 2 * NPAIR * M], mybir.dt.bfloat16, isOutput=False)
    aux = nc.declare_dram_parameter("aux", [2, 576], mybir.dt.bfloat16, isOutput=False)
    out = nc.declare_dram_parameter("out", [128, NA], mybir.dt.float32, isOutput=True)

    A = mybir.AluOpType
    AF = mybir.ActivationFunctionType
    BF = mybir.dt.bfloat16

    sv_idx = {o: i + 1 for i, o in enumerate(DVE_SEQ)}

    from contextlib import ExitStack

    with ExitStack() as stack:
        ec = stack.enter_context
        xb = ec(nc.sbuf_tensor("xb", [128, 2 * NPAIR * M], mybir.dt.bfloat16))
        btw = {
            "A": ec(nc.sbuf_tensor("btwA", [128, NSH2], mybir.dt.uint16)),
            "B": ec(nc.sbuf_tensor("btwB", [128, NSH2], mybir.dt.uint16)),
            "C": ec(nc.sbuf_tensor("btwC", [128, NSH2], mybir.dt.uint16)),
        }
        u = ec(nc.sbuf_tensor("u", [128, NPAIR, NSH2], mybir.dt.uint16))
        auxb = ec(nc.sbuf_tensor("auxb", [2, 576], mybir.dt.bfloat16))
        ot2 = ec(nc.sbuf_tensor("ot2", [128, NA], mybir.dt.float32))
        junk = ec(nc.sbuf_tensor("junk", [128, WARM_N], mybir.dt.bfloat16))
        scr = ec(nc.sbuf_tensor("scr", [1, 1], mybir.dt.float32))
        q0 = ec(nc.psum_tensor("q0", [128, QS[0]], mybir.dt.float32))
        q1 = ec(nc.psum_tensor("q1", [128, QS[1]], mybir.dt.float32))
        psw = ec(nc.psum_tensor("psw", [M, WARM_N], mybir.dt.float32))
        sba = ec(nc.semaphore("sba"))
        sbb = ec(nc.semaphore("sbb"))
        sbc = ec(nc.semaphore("sbc"))
        sq0 = ec(nc.semaphore("sq0"))  # xr0 dma (scalar queue)
        sq1 = ec(nc.semaphore("sq1"))  # aux dma
        sx1 = ec(nc.semaphore("sx1"))  # xr1 dma
        sx2 = ec(nc.semaphore("sx2"))  # xr2 dma
        sx3 = ec(nc.semaphore("sx3"))  # xr3 dma
        sv = ec(nc.semaphore("sv"))  # DVE plane counter
        sa = ec(nc.semaphore("sa"))  # ACT: nop=1, drainQ1=2, drainQ0=3
        sp = ec(nc.semaphore("sp"))  # PE: q1 stopped=1, q0 stopped=2
        so = ec(nc.semaphore("so"))  # out dma completions (2 x 16)
        block = ec(nc.Block())

        @block.sync
        def _(sync: bass.BassEngine):
            sync.dma_start(out=btw["A"][:, :], in_=bpA[:, :]).then_inc(sba, 16)
            sync.dma_start(out=btw["B"][:, :], in_=bpB[:, :]).then_inc(sbb, 16)
            sync.dma_start(out=btw["C"][:, :], in_=bpC[:, :]).then_inc(sbc, 16)
            sync.wait_ge(sa, 2)
            sync.dma_start(out=out[:, QS[0]:NA], in_=ot2[:, QS[0]:NA]).then_inc(so, 16)
            sync.wait_ge(so, 32)

        @block.vector
        def _(vector: bass.BassEngine):
            vector.wait_ge(sba, 16)
            seen = set()
            for o in DVE_SEQ:
                arr, pos, prod = PLANES[o]
                if arr == "B" and "B" not in seen:
                    vector.wait_ge(sbb, 16)
                if arr == "C" and "C" not in seen:
                    vector.wait_ge(sbc, 16)
                seen.add(arr)
                if prod == "isge":
                    vector.tensor_scalar(
                        u[:, o, :].bitcast(BF), btw[arr][:, :], 32768.0, None,
                        op0=A.is_ge,
                    ).then_inc(sv)
                else:
                    vector.tensor_scalar(
                        u[:, o, :], btw[arr][:, :], 1 << pos, None, op0=A.bitwise_and
                    ).then_inc(sv)

        @block.scalar
        def _(scalar: bass.BassEngine):
            scalar.activation(scr[:, :], scr[:, :], AF.Copy, 0.0, 0.0)
            # xr block 0 (planes 0-3) ungated for early PE start
            scalar.dma_start(out=xb[:, 0:512], in_=xr[:, 0:512]).then_inc(sq0, 16)
            scalar.nop().then_inc(sa)  # keep sa numbering: 1 (was sign plane)
            scalar.wait_ge(sp, 1)
            scalar.activation(
                ot2[:, QS[0]:NA], q1[:, :], AF.Copy, bias=0.0, scale=1.0
            ).then_inc(sa)
            scalar.wait_ge(sp, 2)
            scalar.activation(
                ot2[:, 0:QS[0]], q0[:, :], AF.Copy, bias=0.0, scale=1.0
            ).then_inc(sa)
            scalar.dma_start(out=out[:, 0:QS[0]], in_=ot2[:, 0:QS[0]]).then_inc(so, 16)

        @block.gpsimd
        def _(gpsimd: bass.BassEngine):
            gpsimd.dma_start(out=auxb[:, :], in_=aux[:, :]).then_inc(sq1, 16)
            gpsimd.dma_start(out=xb[:, 512:1024], in_=xr[:, 512:1024]).then_inc(sx1, 16)
            gpsimd.dma_start(out=xb[:, 1024:1536], in_=xr[:, 1024:1536]).then_inc(sx2, 16)
            gpsimd.dma_start(out=xb[:, 1536:2048], in_=xr[:, 1536:2048]).then_inc(sx3, 16)

        @block.tensor
        def _(tensor: bass.BassEngine):
            for _i in range(N_WARM):
                tensor.matmul(psw[:, :], junk[:, 0:M], junk[:, :], start=True, stop=True)
            # rank-1 correction initializes psum: psum[m, n] = c_hi[m] + c_lo[m]
            tensor.wait_ge(sq1, 16)
            for tp, pbase in (((0, 0), 0), ((0, 64), 64)):
                tensor.matmul(
                    q0[pbase:pbase + M, :], auxb[0:2, 512:576], auxb[0:2, 0:QS[0]],
                    start=True, stop=False, tile_position=tp,
                )
                tensor.matmul(
                    q1[pbase:pbase + M, :], auxb[0:2, 512:576], auxb[0:2, 0:QS[1]],
                    start=True, stop=False, tile_position=tp,
                )
            for _k in range(3):
                tensor.matmul(
                    psw[:, 0:FILL_N], junk[:, 0:M], junk[:, 0:FILL_N],
                    start=True, stop=True,
                )
            for o in range(NPAIR):
                blk = o // 4
                xr_sem, xr_cnt = [(sq0, 16), (sx1, 16), (sx2, 16), (sx3, 16)][blk]
                tensor.wait_ge(xr_sem, xr_cnt)
                tensor.wait_ge(sv, sv_idx[o])
                is_last = o == NPAIR - 1
                for c in range(NCH):
                    lh = xb[:, (o * 2 + c) * M : (o * 2 + c + 1) * M]
                    tensor.ldweights(lh, tile_position=(0, 0))
                    tensor.ldweights(lh, tile_position=(0, 64))
                    base = c * NSH
                    lc = is_last and c == NCH - 1
                    splits = [(q1, QS[0], QS[1]), (q0, 0, QS[0])] if lc else [
                        (q0, 0, QS[0]), (q1, QS[0], QS[1])]
                    for qt, off, w in splits:
                        for tp, pbase, nbase in (((0, 0), 0, 0), ((0, 64), 64, NA)):
                            mm = tensor.matmul(
                                qt[pbase:pbase + M, :],
                                lh,
                                u[:, o, base + nbase + off : base + nbase + off + w].bitcast(BF),
                                start=False, stop=lc,
                                tile_position=tp,
                            )
                            mm.ins.ldweights = False
                            if lc and tp == (0, 64):
                                mm.then_inc(sp)
                if o < NPAIR - 3:
                    for _k in range(2):
                        tensor.matmul(
                            psw[:, 0:FILL_N], junk[:, 0:M], junk[:, 0:FILL_N],
                            start=True, stop=True,
                        )

    return nc


def _prep(x, bp):
    x = np.asarray(x, dtype=np.float32)
    bp = np.asarray(bp)
    bytes_ = bp.astype(np.uint8)
    B = bytes_.reshape(OUT_F, IN_F // 8)

    # x[m, k] with k = PACK*(128*c + p) + o  ->  xh[p, (o, c, m)], prescaled
    xr4 = np.ascontiguousarray(
        x.reshape(M, NCH, 128, PACK).transpose(2, 3, 1, 0)
    )  # [p, o, c, m]
    for o in range(PACK):
        xr4[:, o, :, :] *= _xscale(o)
    xh = xr4.reshape(128, -1).astype(ml_dtypes.bfloat16)

    # rank-1 correction: every plane is in {0, u_on} form -> -sum over all k
    corr = -x.sum(axis=1)  # [M]
    c_hi = corr.astype(ml_dtypes.bfloat16)
    c_lo = (corr - c_hi.astype(np.float32)).astype(ml_dtypes.bfloat16)
    aux = np.zeros((2, 576), dtype=ml_dtypes.bfloat16)
    aux[:, 0:512] = 1.0
    aux[0, 512:576] = c_hi
    aux[1, 512:576] = c_lo

    in_maps = []
    for cid in range(NCORES):
        Bc = np.ascontiguousarray(B[cid * NSH : (cid + 1) * NSH])  # [1376, 512] u8
        Wd = Bc.view("<u2")  # [1376, NW] little-endian words
        bptT = np.ascontiguousarray(Wd.T)  # [256, 1376], row w = 128c+p
        # bit o of word -> new array/position
        packs = {"A": np.zeros((256, NSH), np.uint16),
                 "B": np.zeros((256, NSH), np.uint16),
                 "C": np.zeros((256, NSH), np.uint16)}
        for o in range(PACK):
            arr, pos, _ = PLANES[o]
            bit = (bptT >> np.uint16(_shift(o))) & np.uint16(1)
            packs[arr] |= bit << np.uint16(pos)
        im = {}
        for name, arrk in (("bpA", "A"), ("bpB", "B"), ("bpC", "C")):
            full = packs[arrk]
            pair = np.concatenate([full[0:128, :], full[128:256, :]], axis=1)
            im[name] = np.ascontiguousarray(pair)
        im["xr"] = xh
        im["aux"] = aux
        in_maps.append(im)
    return in_maps


def _run(x, bp, trace=False):
    in_maps = _prep(x, bp)
    nc = _build()
    res = run_bass_kernel_spmd(nc, in_maps, list(range(NCORES)), trace=trace)
    outs = []
    for c in range(NCORES):
        o = np.asarray(res.results[c]["out"])  # [128, 688]
        outs.append(np.concatenate([o[0:M, :], o[M:128, :]], axis=1))  # [64, 1376]
    full = np.concatenate(outs, axis=1).astype(np.float32)
    return full, res


def kernel(x, bp):
    out, _ = _run(x, bp, trace=False)
    return out


# revision 40
# speedup vs baseline: 1.4701x; 1.0172x over previous
"""BitLinear (1-bit packed weights) on 8 TRN2 NeuronCores — v3.

out = x @ W.T, x [64, 4096] f32, W [11008, 4096] in {-1,+1} unpacked from
bp (one byte per int32, MSB-first bits).

Tensor-parallel over out_features: 8 x 1376 rows of W; x replicated.

v3 design (bitcast unpack):
 - DVE bitvec ops cannot dtype-cast (walrus verifier). Instead, the host
   repacks each weight bit into a bf16 EXPONENT bit position (9..14) of
   uint16 words. Then (word & (1<<s)) reinterpreted (bitcast) as bf16 is
   exactly {0, 2^(2^(s-7)-127)} — a clean one-op-per-plane unpack. The
   weird magnitude is compensated by prescaling the matching x slice by
   2^(128-2^(s-7)) on the host (products are always 2*x*bit).
 - 3 packed arrays btwA/B/C [128, 2752] u16: A carries 7 planes (pos
   15,14,13..9), B 6 (14..9), C 3 (14,13,12). Position 15 is extracted
   by the ACT engine via Sign (gives +-1 directly); the three pos-14
   planes go to GPSIMD tensor_tensor AND (one shared mask); the other 12
   ride the DVE (12 x ~1.04us chain).
 - "-1" of w=2b-1: rank-1 correction initializes psum via a
   contraction-2 matmul (c_hi+c_lo bf16 split for f32 accuracy).
 - Single large DMAs (packets spread over all 16 DMA engines), ungated.
 - u buffer holds all 16 planes: DVE never waits on PE.
 - PE: junk warmup for the p-state ramp, then corr + 16 planes in
   ascending-o order (matches producer completion), junk fillers between.
"""

import sys

sys.path.insert(0, "/opt/trn_rl_repo")

import ml_dtypes
import numpy as np

import concourse.bass as bass
import concourse.mybir as mybir
from concourse.bass_utils import run_bass_kernel_spmd

OUT_F = 11008
IN_F = 4096
M = 64
NCORES = 8
NSH = OUT_F // NCORES  # 1376 rows of W per core
NSH2 = 2 * NSH  # 2752
PACK = 16
NW = IN_F // PACK  # 256 words per W row
NCH = NW // 128  # 2 chunks
NPAIR = PACK  # 16 k-offsets per word
NA = NSH // 2  # 688
QS = (512, 176)  # psum n-splits per half

# plane table: k-offset o -> (array, bit position, producer)
# producers: 'dve', 'act' (Sign, pos must be 15), 'pool' (shared pos-14 mask)
PLANES = {
    0: ("A", 13, "dve"),
    1: ("A", 12, "dve"),
    2: ("A", 15, "sign"),
    3: ("A", 14, "dve"),
    4: ("A", 11, "dve"),
    5: ("A", 10, "dve"),
    6: ("A", 9, "dve"),
    7: ("B", 13, "dve"),
    8: ("B", 12, "dve"),
    9: ("B", 15, "sign"),
    10: ("B", 11, "dve"),
    11: ("B", 10, "dve"),
    12: ("B", 9, "dve"),
    13: ("C", 13, "dve"),
    14: ("C", 15, "sign"),
    15: ("C", 12, "dve"),
}
USE_SIGN = True  # ACT Sign for pos-15 planes; False -> DVE is_ge fallback

if not USE_SIGN:
    PLANES = {
        o: (a, p, "isge" if pr == "sign" else pr) for o, (a, p, pr) in PLANES.items()
    }
DVE_SEQ = [o for o in range(NPAIR) if PLANES[o][2] != "sign"]
SIGN_SEQ = [o for o in range(NPAIR) if PLANES[o][2] == "sign"]
# PE consumption order: sign planes deferred to match ACT completion times
PE_ORDER = [0, 1, 3, 4, 2, 5, 6, 7, 8, 9, 10, 11, 12, 13, 14, 15] if USE_SIGN else list(range(NPAIR))

N_WARM = 7
WARM_N = 256
FILL_N = 192

_np_word = "<u2"  # test.py preflight compat


def _shift(o):
    # ORIGINAL byte packing of bp: bit position of k-offset o
    return 8 * (o // 8) + 7 - (o % 8)


def _xscale(o):
    arr, pos, prod = PLANES[o]
    if prod == "sign":
        return 1.0  # Sign gives +-1 directly
    if prod == "isge":
        return 2.0  # is_ge gives {0,1}
    # u_on = 2^(2^(pos-7)-127); want x'*u_on = 2x
    return float(2.0 ** (128 - (1 << (pos - 7))))


def _build():
    nc = bass.Bass()
    bpA = nc.declare_dram_parameter("bpA", [128, NSH2], mybir.dt.uint16, isOutput=False)
    bpB = nc.declare_dram_parameter("bpB", [128, NSH2], mybir.dt.uint16, isOutput=False)
    bpC = nc.declare_dram_parameter("bpC", [128, NSH2], mybir.dt.uint16, isOutput=False)
    xr = nc.declare_dram_parameter("xr", [128, 2 * NPAIR * M], mybir.dt.bfloat16, isOutput=False)
    aux = nc.declare_dram_parameter("aux", [2, 576], mybir.dt.bfloat16, isOutput=False)
    out = nc.declare_dram_parameter("out", [128, NA], mybir.dt.float32, isOutput=True)

    A = mybir.AluOpType
    AF = mybir.ActivationFunctionType
    BF = mybir.dt.bfloat16

    sv_idx = {o: i + 1 for i, o in enumerate(DVE_SEQ)}

    from contextlib import ExitStack

    with ExitStack() as stack:
        ec = stack.enter_context
        xb = ec(nc.sbuf_tensor("xb", [128, 2 * NPAIR * M], mybir.dt.bfloat16))
        btw = {
            "A": ec(nc.sbuf_tensor("btwA", [128, NSH2], mybir.dt.uint16)),
            "B": ec(nc.sbuf_tensor("btwB", [128, NSH2], mybir.dt.uint16)),
            "C": ec(nc.sbuf_tensor("btwC", [128, NSH2], mybir.dt.uint16)),
        }
        u = ec(nc.sbuf_tensor("u", [128, NPAIR, NSH2], mybir.dt.uint16))
        usg = ec(nc.sbuf_tensor("usg", [128, max(1, len(SIGN_SEQ)), NSH2], mybir.dt.bfloat16))
        auxb = ec(nc.sbuf_tensor("auxb", [2, 576], mybir.dt.bfloat16))
        ot2 = ec(nc.sbuf_tensor("ot2", [128, NA], mybir.dt.float32))
        junk = ec(nc.sbuf_tensor("junk", [128, WARM_N], mybir.dt.bfloat16))
        scr = ec(nc.sbuf_tensor("scr", [1, 1], mybir.dt.float32))
        q0 = ec(nc.psum_tensor("q0", [128, QS[0]], mybir.dt.float32))
        q1 = ec(nc.psum_tensor("q1", [128, QS[1]], mybir.dt.float32))
        psw = ec(nc.psum_tensor("psw", [M, WARM_N], mybir.dt.float32))
        sba = ec(nc.semaphore("sba"))
        sbb = ec(nc.semaphore("sbb"))
        sbc = ec(nc.semaphore("sbc"))
        sq0 = ec(nc.semaphore("sq0"))  # xr0 dma (scalar queue)
        sq1 = ec(nc.semaphore("sq1"))  # aux dma
        sx1 = ec(nc.semaphore("sx1"))  # xr1 dma
        sx2 = ec(nc.semaphore("sx2"))  # xr2 dma
        sx3 = ec(nc.semaphore("sx3"))  # xr3 dma
        sv = ec(nc.semaphore("sv"))  # DVE plane counter
        sa = ec(nc.semaphore("sa"))  # ACT: signA=1 signB=2 signC=3 drQ1=4 drQ0a=5
        sg = ec(nc.semaphore("sg"))  # gpsimd: sign-bias memset = 1
        sp = ec(nc.semaphore("sp"))  # PE: q1 stopped=1, q0 stopped=2
        sd2 = ec(nc.semaphore("sd2"))  # DVE drain piece done
        so = ec(nc.semaphore("so"))  # out dma completions (2 x 16)
        bsgn = ec(nc.sbuf_tensor("bsgn", [128, 1], mybir.dt.float32))
        block = ec(nc.Block())

        # btw arrays split L/R across two queues each -> sb* waits are >= 32
        HALF = NSH  # 1376 cols

        @block.sync
        def _(sync: bass.BassEngine):
            sync.dma_start(out=btw["A"][:, 0:HALF], in_=bpA[:, 0:HALF]).then_inc(sba, 16)
            sync.dma_start(out=btw["B"][:, 0:HALF], in_=bpB[:, 0:HALF]).then_inc(sbb, 16)
            sync.dma_start(out=btw["C"][:, 0:HALF], in_=bpC[:, 0:HALF]).then_inc(sbc, 16)
            sync.wait_ge(sa, len(SIGN_SEQ) + 1)
            sync.dma_start(out=out[:, QS[0]:NA], in_=ot2[:, QS[0]:NA]).then_inc(so, 16)
            sync.wait_ge(so, 32)

        @block.vector
        def _(vector: bass.BassEngine):
            vector.wait_ge(sba, 32)
            seen = set()
            for o in DVE_SEQ:
                arr, pos, prod = PLANES[o]
                if arr == "B" and "B" not in seen:
                    vector.wait_ge(sbb, 32)
                if arr == "C" and "C" not in seen:
                    vector.wait_ge(sbc, 32)
                seen.add(arr)
                if prod == "isge":
                    vector.tensor_scalar(
                        u[:, o, :].bitcast(BF), btw[arr][:, :], 32768.0, None,
                        op0=A.is_ge,
                    ).then_inc(sv)
                else:
                    vector.tensor_scalar(
                        u[:, o, :], btw[arr][:, :], 1 << pos, None, op0=A.bitwise_and
                    ).then_inc(sv)


        @block.scalar
        def _(scalar: bass.BassEngine):
            scalar.dma_start(out=btw["A"][:, HALF:NSH2], in_=bpA[:, HALF:NSH2]).then_inc(sba, 16)
            # xr block 0 (planes 0-3) ungated for early PE start
            scalar.dma_start(out=xb[:, 0:512], in_=xr[:, 0:512]).then_inc(sq0, 16)
            scalar.dma_start(out=btw["C"][:, HALF:NSH2], in_=bpC[:, HALF:NSH2]).then_inc(sbc, 16)
            scalar.activation(scr[:, :], scr[:, :], AF.Copy, 0.0, 0.0)  # table prime
            scalar.wait_ge(sg, 1)
            for i, o in enumerate(SIGN_SEQ):
                arr, pos, _ = PLANES[o]
                scalar.wait_ge({"A": sba, "B": sbb, "C": sbc}[arr], 32)
                scalar.activation(
                    usg[:, i, :], btw[arr][:, :], AF.Sign,
                    bias=bsgn[:, 0:1], scale=1.0,
                ).then_inc(sa)
            scalar.wait_ge(sp, 1)
            scalar.activation(
                ot2[:, QS[0]:NA], q1[:, :], AF.Copy, bias=0.0, scale=1.0
            ).then_inc(sa)
            scalar.wait_ge(sp, 2)
            scalar.activation(
                ot2[:, 0:QS[0]], q0[:, :], AF.Copy, bias=0.0, scale=1.0
            ).then_inc(sa)
            scalar.dma_start(out=out[:, 0:QS[0]], in_=ot2[:, 0:QS[0]]).then_inc(so, 16)

        @block.gpsimd
        def _(gpsimd: bass.BassEngine):
            gpsimd.memset(bsgn[:, :], -32767.5).then_inc(sg)
            gpsimd.dma_start(out=auxb[:, :], in_=aux[:, :]).then_inc(sq1, 16)
            gpsimd.dma_start(out=btw["B"][:, HALF:NSH2], in_=bpB[:, HALF:NSH2]).then_inc(sbb, 16)
            gpsimd.dma_start(out=xb[:, 512:1024], in_=xr[:, 512:1024]).then_inc(sx1, 16)
            gpsimd.dma_start(out=xb[:, 1024:1536], in_=xr[:, 1024:1536]).then_inc(sx2, 16)
            gpsimd.dma_start(out=xb[:, 1536:2048], in_=xr[:, 1536:2048]).then_inc(sx3, 16)

        @block.tensor
        def _(tensor: bass.BassEngine):
            for _i in range(N_WARM):
                tensor.matmul(psw[:, :], junk[:, 0:M], junk[:, :], start=True, stop=True)
            # rank-1 correction initializes psum: psum[m, n] = c_hi[m] + c_lo[m]
            tensor.wait_ge(sq1, 16)
            for tp, pbase in (((0, 0), 0), ((0, 64), 64)):
                tensor.matmul(
                    q0[pbase:pbase + M, :], auxb[0:2, 512:576], auxb[0:2, 0:QS[0]],
                    start=True, stop=False, tile_position=tp,
                )
                tensor.matmul(
                    q1[pbase:pbase + M, :], auxb[0:2, 512:576], auxb[0:2, 0:QS[1]],
                    start=True, stop=False, tile_position=tp,
                )
            for _k in range(2):
                tensor.matmul(
                    psw[:, 0:FILL_N], junk[:, 0:M], junk[:, 0:FILL_N],
                    start=True, stop=True,
                )
            sa_idx = {o: i + 1 for i, o in enumerate(SIGN_SEQ)}
            for idx, o in enumerate(PE_ORDER):
                blk = o // 4
                xr_sem, xr_cnt = [(sq0, 16), (sx1, 16), (sx2, 16), (sx3, 16)][blk]
                tensor.wait_ge(xr_sem, xr_cnt)
                is_sign = PLANES[o][2] == "sign"
                if is_sign:
                    tensor.wait_ge(sa, sa_idx[o])
                else:
                    tensor.wait_ge(sv, sv_idx[o])
                is_last = idx == NPAIR - 1
                for c in range(NCH):
                    lh = xb[:, (o * 2 + c) * M : (o * 2 + c + 1) * M]
                    tensor.ldweights(lh, tile_position=(0, 0))
                    tensor.ldweights(lh, tile_position=(0, 64))
                    base = c * NSH
                    lc = is_last and c == NCH - 1
                    splits = [(q1, QS[0], QS[1]), (q0, 0, QS[0])] if lc else [
                        (q0, 0, QS[0]), (q1, QS[0], QS[1])]
                    for qt, off, w in splits:
                        for tp, pbase, nbase in (((0, 0), 0, 0), ((0, 64), 64, NA)):
                            lo = base + nbase + off
                            rhs = (
                                usg[:, sa_idx[o] - 1, lo : lo + w]
                                if is_sign
                                else u[:, o, lo : lo + w].bitcast(BF)
                            )
                            mm = tensor.matmul(
                                qt[pbase:pbase + M, :],
                                lh,
                                rhs,
                                start=False, stop=lc,
                                tile_position=tp,
                            )
                            mm.ins.ldweights = False
                            if lc and tp == (0, 64):
                                mm.then_inc(sp)
                if idx < NPAIR - 2:
                    tensor.matmul(
                        psw[:, 0:FILL_N], junk[:, 0:M], junk[:, 0:FILL_N],
                        start=True, stop=True,
                    )

    return nc


def _prep(x, bp):
    x = np.asarray(x, dtype=np.float32)
    bp = np.asarray(bp)
    bytes_ = bp.astype(np.uint8)
    B = bytes_.reshape(OUT_F, IN_F // 8)

    # x[m, k] with k = PACK*(128*c + p) + o  ->  xh[p, (o, c, m)], prescaled
    xr4 = np.ascontiguousarray(
        x.reshape(M, NCH, 128, PACK).transpose(2, 3, 1, 0)
    )  # [p, o, c, m]
    for o in range(PACK):
        xr4[:, o, :, :] *= _xscale(o)
    xh = xr4.reshape(128, -1).astype(ml_dtypes.bfloat16)

    # rank-1 correction: {0, u_on}-form planes need -sum(x) over their k's;
    # Sign planes (+-1 form) need none.
    ks = x.reshape(M, NW, PACK)
    sign_sum = sum((ks[:, :, o].sum(axis=1) for o in SIGN_SEQ), np.zeros(M, np.float32))
    corr = -(x.sum(axis=1) - sign_sum)  # [M]
    c_hi = corr.astype(ml_dtypes.bfloat16)
    c_lo = (corr - c_hi.astype(np.float32)).astype(ml_dtypes.bfloat16)
    aux = np.zeros((2, 576), dtype=ml_dtypes.bfloat16)
    aux[:, 0:512] = 1.0
    aux[0, 512:576] = c_hi
    aux[1, 512:576] = c_lo

    in_maps = []
    for cid in range(NCORES):
        Bc = np.ascontiguousarray(B[cid * NSH : (cid + 1) * NSH])  # [1376, 512] u8
        Wd = Bc.view("<u2")  # [1376, NW] little-endian words
        bptT = np.ascontiguousarray(Wd.T)  # [256, 1376], row w = 128c+p
        # bit o of word -> new array/position
        packs = {"A": np.zeros((256, NSH), np.uint16),
                 "B": np.zeros((256, NSH), np.uint16),
                 "C": np.zeros((256, NSH), np.uint16)}
        for o in range(PACK):
            arr, pos, _ = PLANES[o]
            bit = (bptT >> np.uint16(_shift(o))) & np.uint16(1)
            packs[arr] |= bit << np.uint16(pos)
        im = {}
        for name, arrk in (("bpA", "A"), ("bpB", "B"), ("bpC", "C")):
            full = packs[arrk]
            pair = np.concatenate([full[0:128, :], full[128:256, :]], axis=1)
            im[name] = np.ascontiguousarray(pair)
        im["xr"] = xh
        im["aux"] = aux
        in_maps.append(im)
    return in_maps


def _run(x, bp, trace=False):
    in_maps = _prep(x, bp)
    nc = _build()
    res = run_bass_kernel_spmd(nc, in_maps, list(range(NCORES)), trace=trace)
    outs = []
    for c in range(NCORES):
        o = np.asarray(res.results[c]["out"])  # [128, 688]
        outs.append(np.concatenate([o[0:M, :], o[M:128, :]], axis=1))  # [64, 1376]
    full = np.concatenate(outs, axis=1).astype(np.float32)
    return full, res


def kernel(x, bp):
    out, _ = _run(x, bp, trace=False)
    return out
